# revision 4
# baseline (speedup 1.0000x reference)
"""Sliding-window attention (BERT-style, window +/-256, RoPE) on 8 TRN2 NeuronCores.

Sharding: core c -> batch b = c//4, head-group g = c%4 (4 of 16 heads each).
Per (head, 128-query-block) unit: banded f16 QK^T scores (8.0 = sqrt(HD)
folded into Wq on host) with the band mask added in-PSUM via identity
matmuls, DVE row-max, ACT exp (bias = -rowmax) -> bf16 P, PE transpose of P,
DVE eviction, P^T V with an appended ones-column. Numerator and denominator
are DMA'd out; the division happens on the host during unsharding.

The emission is software-pipelined (scores/rowmax/exp | transpose/evict | PV
skewed across units) and the V / second-half Q,K projections are paced into
the attention unit stream as PE filler; input DMAs are split so the first
K-projection starts during the hidden-states load.
"""
import os
import sys

sys.path.insert(0, "/opt/trn_rl_repo")

import numpy as np
import ml_dtypes

import concourse.bass as bass
import concourse.mybir as mybir
import concourse.tile as tile
from concourse import bacc
from concourse.bass_utils import run_bass_kernel_spmd

F16 = mybir.dt.float16
BF16 = mybir.dt.bfloat16
F32 = mybir.dt.float32
AF = mybir.ActivationFunctionType
ALU = mybir.AluOpType

B, S, D, H, HD = 2, 2048, 1024, 16, 64
WIN = 256
NSTRIP = 640
NQB = S // 128        # 16 query blocks
HPC = 4               # heads per core
HDPC = HPC * HD       # 256 output dims per core
ROPE_THETA = 10000.0

LAST_EXEC_NS = None
LAST_RESULTS = None
EVICT_MODE = "dve"


def strip_start(qb):
    return min(max(qb * 128 - WIN, 0), S - NSTRIP)


def qb_info(qb):
    """Per query block: (c0v, c1v, jb0, jb1) valid column range and valid
    transposed-block range within the 640-wide strip."""
    s0 = strip_start(qb)
    i0 = qb * 128
    cols = np.arange(s0, s0 + NSTRIP)
    qq = np.arange(i0, i0 + 128)
    valid = (cols[None, :] >= qq[:, None] - WIN) & (cols[None, :] <= qq[:, None] + WIN)
    anyv = valid.any(axis=0)
    c0v = int(np.argmax(anyv))
    c1v = NSTRIP - int(np.argmax(anyv[::-1]))
    jb0, jb1 = c0v // 128, (c1v + 127) // 128
    assert c0v % 128 == 0 and c1v % 128 == 0
    return c0v, c1v, jb0, jb1


MASK_VAL = -60000.0


def mask_info():
    """Per qb: (c0, c1, off) column segments inside the valid window that
    contain out-of-band cells (split at the 512 PSUM bank boundary), plus the
    packed [128, total] f16 additive-mask tensor."""
    segs_per_qb = []
    cols = []
    total = 0
    for qb in range(NQB):
        i0 = qb * 128
        s0 = strip_start(qb)
        c0v, c1v, _, _ = qb_info(qb)
        ql = np.arange(i0, i0 + 128)[:, None]
        kk = np.arange(s0, s0 + NSTRIP)[None, :]
        valid = (kk >= ql - WIN) & (kk <= ql + WIN)
        bad_col = (~valid).any(axis=0)
        bad_col[:c0v] = False
        bad_col[c1v:] = False
        runs = []
        c = c0v
        while c < c1v:
            if bad_col[c]:
                c1 = c
                while c1 < c1v and bad_col[c1]:
                    c1 += 1
                if c < 512 < c1:
                    runs.append((c, 512))
                    runs.append((512, c1))
                else:
                    runs.append((c, c1))
                c = c1
            else:
                c += 1
        seg_list = []
        for (c0, c1) in runs:
            m = np.where(valid[:, c0:c1], np.float32(0.0), np.float32(MASK_VAL))
            cols.append(m)
            seg_list.append((c0, c1, total))
            total += c1 - c0
        segs_per_qb.append(seg_list)
    packed = np.concatenate(cols, axis=1).astype(np.float16)
    return segs_per_qb, packed


def rope_tables():
    inv_freq = 1.0 / (ROPE_THETA ** (np.arange(0, HD, 2, dtype=np.float32) / HD))
    t = np.arange(S, dtype=np.float32)
    freqs = np.outer(t, inv_freq)                      # [S, 32]
    emb = np.concatenate([freqs, freqs], axis=-1)      # [S, 64]
    cos = np.cos(emb)
    sin = np.sin(emb)
    cosT = np.tile(cos.T, (2, 1))                      # [128, S]
    sinT = np.tile(sin.T, (2, 1))
    sign = np.where((np.arange(128) % 64) < 32, -1.0, 1.0)[:, None]
    return cosT.astype(np.float16), (sinT * sign).astype(np.float16)


MASK_SEGS, MASK_PACKED = mask_info()
MASK_COLS = MASK_PACKED.shape[1]

_NC_CACHE = None


def build(body_reps=1, share=True, **kwargs):
    nc = bacc.Bacc("TRN2", target_bir_lowering=False, debug=False, num_devices=8)
    xt_d = nc.dram_tensor("xt", [D, S], F16, kind="ExternalInput").ap()
    wq_d = nc.dram_tensor("wq", [D, HDPC], F16, kind="ExternalInput").ap()
    wk_d = nc.dram_tensor("wk", [D, HDPC], F16, kind="ExternalInput").ap()
    wv_d = nc.dram_tensor("wv", [D, HDPC], F16, kind="ExternalInput").ap()
    cos_d = nc.dram_tensor("cosr", [128, S], F16, kind="ExternalInput").ap()
    sin_d = nc.dram_tensor("sinr", [128, S], F16, kind="ExternalInput").ap()
    msk_d = nc.dram_tensor("msk", [128, MASK_COLS], F16, kind="ExternalInput").ap()
    id16_d = nc.dram_tensor("id16", [128, 128], F16, kind="ExternalInput").ap()
    idbf_d = nc.dram_tensor("idbf", [128, 128], BF16, kind="ExternalInput").ap()
    out_d = nc.dram_tensor("out", [S, HPC, HD + 1], F32, kind="ExternalOutput").ap()

    with tile.TileContext(nc) as tc:
        with (
            tc.tile_pool(name="const", bufs=1) as cpool,
            tc.tile_pool(name="qk", bufs=1) as qkpool,
            tc.tile_pool(name="scratch", bufs=2) as spool,
            tc.tile_pool(name="attn", bufs=6) as apool,
            tc.tile_pool(name="small", bufs=12) as smpool,
            tc.tile_pool(name="ps", bufs=2, space="PSUM") as ps,
        ):
            # ---- loads (ordered so the K projection can start ASAP) ----
            w_sb = {}
            for nm, d in (("wk", wk_d), ("wq", wq_d), ("wv", wv_d)):
                t = cpool.tile([128, 8, HDPC], F16, name=nm + "_sb")
                w_sb[nm] = t
            xt_sb = cpool.tile([128, 8, S], F16, name="xt_sb")
            xt_r = xt_d.rearrange("(kt p) s -> p kt s", p=128)
            nc.sync.dma_start(w_sb["wk"][:], wk_d.rearrange("(kt p) m -> p kt m", p=128))
            for xc in range(8):
                nc.sync.dma_start(xt_sb[:, :, xc * 256:(xc + 1) * 256],
                                  xt_r[:, :, xc * 256:(xc + 1) * 256])
                if xc == 1:
                    nc.sync.dma_start(w_sb["wq"][:],
                                      wq_d.rearrange("(kt p) m -> p kt m", p=128))
                if xc == 3:
                    nc.sync.dma_start(w_sb["wv"][:],
                                      wv_d.rearrange("(kt p) m -> p kt m", p=128))
            cos_sb = cpool.tile([128, S], F16, name="cos_sb")
            nc.sync.dma_start(cos_sb[:], cos_d)
            sin_sb = cpool.tile([128, S], F16, name="sin_sb")
            nc.sync.dma_start(sin_sb[:], sin_d)
            msk_sb = cpool.tile([128, MASK_COLS], F16, name="msk_sb")
            nc.sync.dma_start(msk_sb[:], msk_d)
            id16_sb = cpool.tile([128, 128], F16, name="id16_sb")
            nc.sync.dma_start(id16_sb[:], id16_d)
            idbf_sb = cpool.tile([128, 128], BF16, name="idbf_sb")
            nc.sync.dma_start(idbf_sb[:], idbf_d)

            saved = {}
            for rep in range(body_reps):
                if rep > 0 and share:
                    v_sb = saved["v_sb"]
                    qk_t = saved["qk_t"]
                else:
                    v_sb = cpool.tile([128, NQB, HPC, HD + 1], BF16,
                                      tag="v_sb" if share else f"r{rep}v_sb",
                                      name=f"r{rep}v_sb")
                    nc.vector.memset(v_sb[:, :, :, HD:HD + 1], 1.0)

                    def v_chunk(sb):
                        vps = ps.tile([128, 512], F32, tag="pj", bufs=2, name=f"r{rep}vps{sb}")
                        for kt in range(8):
                            nc.tensor.matmul(vps[:, 0:HDPC],
                                             xt_sb[:, kt, sb * 128:(sb + 1) * 128],
                                             w_sb["wv"][:, kt, :],
                                             start=(kt == 0), stop=(kt == 7))
                        nc.scalar.activation(
                            v_sb[:, sb, :, 0:HD],
                            vps[:, 0:HDPC].rearrange("p (h c) -> p h c", h=HPC),
                            AF.Copy)

                    qk_t = {}
                    qk_state = {}

                    def qk_mm_chunk(nm, m, sc_i):
                        if (nm, m) not in qk_state:
                            qk_state[(nm, m)] = spool.tile(
                                [128, S], F16, tag="rope_raw", name=f"r{rep}{nm}raw{m}")
                        raw = qk_state[(nm, m)]
                        pps = ps.tile([128, 512], F32, tag="pj", bufs=2,
                                      name=f"r{rep}{nm}ps{m}_{sc_i}")
                        for kt in range(8):
                            nc.tensor.matmul(
                                pps[:, 0:512],
                                w_sb["w" + nm][:, kt, m * 128:(m + 1) * 128],
                                xt_sb[:, kt, sc_i * 512:(sc_i + 1) * 512],
                                start=(kt == 0), stop=(kt == 7))
                        nc.scalar.activation(raw[:, sc_i * 512:(sc_i + 1) * 512],
                                             pps[:, 0:512], AF.Copy)

                    def qk_rope_chunk(nm, m):
                        raw = qk_state.pop((nm, m))
                        rot = spool.tile([128, S], F16, tag="rope_rot",
                                         name=f"r{rep}{nm}rot{m}")
                        t1 = spool.tile([128, S], F16, tag="rope_t1",
                                        name=f"r{rep}{nm}t1_{m}")
                        t2 = spool.tile([128, S], F16, tag="rope_t2",
                                        name=f"r{rep}{nm}t2_{m}")
                        dst = qkpool.tile([128, S], F16,
                                          tag=f"qk_{nm}_{m}" if share else f"r{rep}qk_{nm}_{m}",
                                          name=f"r{rep}{nm}_sb{m}")
                        for rc in range(4):
                            cs = slice(rc * 512, (rc + 1) * 512)
                            for gg in range(2):
                                b0 = 64 * gg
                                nc.sync.dma_start(rot[b0:b0 + 32, cs],
                                                  raw[b0 + 32:b0 + 64, cs])
                                nc.sync.dma_start(rot[b0 + 32:b0 + 64, cs],
                                                  raw[b0:b0 + 32, cs])
                            teng = nc.vector if m == 0 else nc.gpsimd
                            nc.vector.tensor_tensor(out=t1[:, cs], in0=raw[:, cs],
                                                    in1=cos_sb[:, cs], op=ALU.mult)
                            teng.tensor_tensor(out=t2[:, cs], in0=rot[:, cs],
                                               in1=sin_sb[:, cs], op=ALU.mult)
                            nc.vector.tensor_tensor(out=dst[:, cs], in0=t1[:, cs],
                                                    in1=t2[:, cs], op=ALU.add)
                        qk_t[(nm, m)] = dst

                    # QK m=0 up front (K first: attention A(qb0) needs the
                    # whole first K strip but only the first Q chunk); V and
                    # QK m=1 become filler chunks paced into the unit stream.
                    for nm in ("k", "q"):
                        for sc_i in range(4):
                            qk_mm_chunk(nm, 0, sc_i)
                        qk_rope_chunk(nm, 0)
                    v_chunks = [(lambda sb=sb: v_chunk(sb)) for sb in range(NQB)]
                    qk1_chunks = []
                    for nm in ("q", "k"):
                        for sc_i in range(4):
                            qk1_chunks.append(
                                lambda nm=nm, sc_i=sc_i: qk_mm_chunk(nm, 1, sc_i))
                        qk1_chunks.append(lambda nm=nm: qk_rope_chunk(nm, 1))
                    saved["v_sb"] = v_sb
                    saved["qk_t"] = qk_t

                # ---- attention: 4 heads x 16 query blocks, software-pipelined
                # emission with 1-unit skew per stage so no engine's compiled
                # order head-of-line blocks on a later stage ----
                units = []
                for hp2 in range(2):
                    for qb in range(NQB):
                        for hh in range(2):
                            units.append((hp2 * 2 + hh, qb))
                NU = len(units)
                state = {}

                def stage_a(i):
                    h, qb = units[i]
                    m, hp = h // 2, 64 * (h % 2)
                    qs, ks = qk_t[("q", m)], qk_t[("k", m)]
                    s0 = strip_start(qb)
                    c0v, c1v, jb0, jb1 = qb_info(qb)
                    scp = ps.tile([128, NSTRIP], F32, tag="sc", name=f"r{rep}sc{h}_{qb}")
                    segs = MASK_SEGS[qb]
                    for (ca, cb) in ((c0v, min(c1v, 512)), (512, c1v)):
                        if cb <= ca:
                            continue
                        rsegs = [sg for sg in segs if ca <= sg[0] < cb]
                        nc.tensor.matmul(scp[:, ca:cb],
                                         qs[hp:hp + 64, qb * 128:(qb + 1) * 128],
                                         ks[hp:hp + 64, s0 + ca:s0 + cb],
                                         start=True, stop=not rsegs)
                        for si, (c0, c1, off) in enumerate(rsegs):
                            nc.tensor.matmul(scp[:, c0:c1], id16_sb[:],
                                             msk_sb[:, off:off + (c1 - c0)],
                                             start=False, stop=(si == len(rsegs) - 1),
                                             skip_group_check=True)
                    negmax = smpool.tile([128, 1], F32, tag="negmax",
                                         name=f"r{rep}nm{h}_{qb}")
                    nc.vector.tensor_reduce(out=negmax[:], in_=scp[:, c0v:c1v],
                                            axis=mybir.AxisListType.X,
                                            op=ALU.max, negate=True)
                    p_t = apool.tile([128, NSTRIP], BF16, tag="p", name=f"r{rep}p{h}_{qb}")
                    nc.scalar.activation(p_t[:, c0v:c1v], scp[:, c0v:c1v], AF.Exp,
                                         bias=negmax[:], scale=1.0)
                    state[(i, "p")] = p_t

                def stage_b(i):
                    h, qb = units[i]
                    c0v, c1v, jb0, jb1 = qb_info(qb)
                    p_t = state.pop((i, "p"))
                    ptp = ps.tile([128, NSTRIP], BF16, tag="ptp", bufs=1, name=f"r{rep}ptp{h}_{qb}")
                    for j in range(jb0, jb1):
                        nc.tensor.transpose(ptp[:, j * 128:(j + 1) * 128],
                                            p_t[:, j * 128:(j + 1) * 128], idbf_sb[:])
                    pts = apool.tile([128, NSTRIP], BF16, tag="pts", name=f"r{rep}pts{h}_{qb}")
                    if EVICT_MODE == "phase":
                        eng = nc.vector if (i < 32 or i % 2 == 0) else nc.gpsimd
                    else:
                        eng = nc.vector
                    eng.tensor_copy(pts[:, c0v:c1v], ptp[:, c0v:c1v])
                    state[(i, "pts")] = pts

                def stage_c(i):
                    h, qb = units[i]
                    s0 = strip_start(qb)
                    c0v, c1v, jb0, jb1 = qb_info(qb)
                    qg, qt = qb // 4, qb % 4
                    pts = state.pop((i, "pts"))
                    if qt == 0:
                        state[(h, qg, "ctx")] = ps.tile([128, 4, HD + 1], F32, tag="ctx", bufs=1,
                                                        name=f"r{rep}ctx{h}_{qg}")
                    ctx = state[(h, qg, "ctx")]
                    for j in range(jb0, jb1):
                        nc.tensor.matmul(ctx[:, qt, :],
                                         pts[:, j * 128:(j + 1) * 128],
                                         v_sb[:, s0 // 128 + j, h, :],
                                         start=(j == jb0), stop=(j == jb1 - 1))
                    if qt == 3:
                        ctx = state.pop((h, qg, "ctx"))
                        o_t = smpool.tile([128, 4, HD + 1], F32, tag="o",
                                          name=f"r{rep}o{h}_{qg}")
                        nc.scalar.activation(o_t[:], ctx[:], AF.Copy)
                        nc.sync.dma_start(
                            out_d[qg * 512:(qg + 1) * 512, h, :].rearrange(
                                "(t p) c -> p t c", p=128), o_t[:])

                if rep > 0 and share:
                    v_chunks, qk1_chunks = [], []
                emitted_v = 0
                for i in range(NU + 2):
                    if i < NU:
                        stage_a(i)
                    # pace fillers: V chunk sb needed by C-stage of qb=sb-2
                    # (unit 2qb+2 at iter +2); all QK1 chunks before unit 32.
                    need_v = min(len(v_chunks) + emitted_v, max(0, (i - 2) // 2 + 3))
                    if v_chunks and emitted_v < need_v:
                        v_chunks.pop(0)()
                        emitted_v += 1
                    if qk1_chunks and (i >= 8 and i % 2 == 1 or i >= 22):
                        qk1_chunks.pop(0)()
                    if 1 <= i < NU + 1:
                        stage_b(i - 1)
                    if i >= 2:
                        stage_c(i - 2)
    nc.compile()
    return nc


def kernel(hidden_states, attention_mask, Wq, bq, Wk, bk, Wv, bv):
    global _NC_CACHE, LAST_EXEC_NS, LAST_RESULTS
    hidden_states = np.asarray(hidden_states, dtype=np.float32)
    attention_mask = np.asarray(attention_mask)
    Wq = np.asarray(Wq, dtype=np.float32)
    Wk = np.asarray(Wk, dtype=np.float32)
    Wv = np.asarray(Wv, dtype=np.float32)
    for bias in (bq, bk, bv):
        assert np.all(np.asarray(bias) == 0.0), "nonzero biases unsupported"

    cosT, sinT = rope_tables()
    idbf = np.eye(128, dtype=np.float32).astype(ml_dtypes.bfloat16)
    id16 = np.eye(128, dtype=np.float16)

    xt16 = [np.ascontiguousarray(hidden_states[b].T).astype(np.float16) for b in range(B)]
    in_maps = []
    for c in range(8):
        b, g = c // 4, c % 4
        sl = slice(g * HDPC, (g + 1) * HDPC)
        in_maps.append(dict(
            xt=xt16[b],
            wq=np.ascontiguousarray((Wq[sl, :] * 8.0).T).astype(np.float16),
            wk=np.ascontiguousarray(Wk[sl, :].T).astype(np.float16),
            wv=np.ascontiguousarray(Wv[sl, :].T).astype(np.float16),
            cosr=cosT, sinr=sinT, msk=MASK_PACKED, id16=id16, idbf=idbf,
        ))

    if _NC_CACHE is None:
        _NC_CACHE = build()
    trace = bool(int(os.environ.get("KERNEL_TRACE", "0")))
    res = run_bass_kernel_spmd(_NC_CACHE, in_maps, core_ids=list(range(8)),
                               trace=trace)
    LAST_EXEC_NS = res.exec_time_ns
    LAST_RESULTS = res

    out = np.empty((B, S, D), np.float32)
    for c in range(8):
        b, g = c // 4, c % 4
        o = np.asarray(res.results[c]["out"], np.float32)   # [S, HPC, HD+1]
        out[b, :, g * HDPC:(g + 1) * HDPC] = (
            o[:, :, :HD] / o[:, :, HD:HD + 1]).reshape(S, HDPC)
    qmask = (np.asarray(attention_mask) > 0).astype(np.float32)[:, :, None]
    return out * qmask


def bench(in_maps, warmup=3, iters=30, nc_override=None):
    """Time repeated executions of the compiled 8-core kernel with inputs
    kept on device. Returns avg seconds per call (upper bound on HW time:
    includes dispatch)."""
    import time
    import jax
    from jax.sharding import Mesh, PartitionSpec
    from jax.experimental.shard_map import shard_map
    from concourse import bass2jax
    from concourse.bass2jax import _bass_exec_p, partition_id_tensor, install_neuronx_cc_hook

    global _NC_CACHE
    if nc_override is not None:
        nc = nc_override
    else:
        if _NC_CACHE is None:
            _NC_CACHE = build()
        nc = _NC_CACHE
    install_neuronx_cc_hook()
    n_cores = 8
    partition_name = nc.partition_id_tensor.name if nc.partition_id_tensor else None
    in_names, out_names, out_avals, zero_outs = [], [], [], []
    for alloc in nc.m.functions[0].allocations:
        if not isinstance(alloc, mybir.MemoryLocationSet):
            continue
        name = alloc.memorylocations[0].name
        if alloc.kind == "ExternalInput":
            if name != partition_name:
                in_names.append(name)
        elif alloc.kind == "ExternalOutput":
            out_names.append(name)
            shape = tuple(alloc.tensor_shape)
            dtype = mybir.dt.np(alloc.dtype)
            out_avals.append(jax.core.ShapedArray(shape, dtype))
            zero_outs.append(np.zeros(shape, dtype))
    n_params = len(in_names)
    n_outs = len(out_avals)
    all_names = in_names + out_names + ([partition_name] if partition_name else [])

    def _body(*args):
        operands = list(args)
        if partition_name is not None:
            operands.append(partition_id_tensor())
        outs = _bass_exec_p.bind(
            *operands, out_avals=tuple(out_avals), in_names=tuple(all_names),
            out_names=tuple(out_names), lowering_input_output_aliases=(),
            sim_require_finite=True, sim_require_nnan=True, nc=nc)
        return tuple(outs)

    devices = jax.devices()[:n_cores]
    mesh = Mesh(np.asarray(devices), ("core",))
    donate = tuple(range(n_params, n_params + n_outs))
    sharded = jax.jit(
        shard_map(_body, mesh=mesh, in_specs=(PartitionSpec("core"),) * (n_params + n_outs),
                  out_specs=(PartitionSpec("core"),) * n_outs, check_rep=False),
        donate_argnums=donate, keep_unused=True)
    concat_in = [np.concatenate([np.asarray(in_maps[c][nm]) for c in range(n_cores)], axis=0)
                 for nm in in_names]
    import jax.numpy as jnp
    sharding = jax.sharding.NamedSharding(mesh, PartitionSpec("core"))
    dev_in = [jax.device_put(a, sharding) for a in concat_in]

    def fresh_zeros():
        return [jax.device_put(np.zeros((n_cores * z.shape[0], *z.shape[1:]), z.dtype), sharding)
                for z in zero_outs]

    for _ in range(warmup):
        outs = sharded(*dev_in, *fresh_zeros())
        jax.block_until_ready(outs)
    zsets = [fresh_zeros() for _ in range(iters)]
    jax.block_until_ready(zsets)
    t0 = time.time()
    all_outs = []
    for i in range(iters):
        all_outs.append(sharded(*dev_in, *zsets[i]))
    jax.block_until_ready(all_outs)
    t1 = time.time()
    return (t1 - t0) / iters


def bench_many(in_maps, ncs, warmup=3, iters=40):
    """Interleaved round-robin timing of multiple compiled kernels.
    Returns list of avg seconds per call, drift-robust."""
    import time
    import jax
    from jax.sharding import Mesh, PartitionSpec
    from jax.experimental.shard_map import shard_map
    from concourse.bass2jax import _bass_exec_p, partition_id_tensor, install_neuronx_cc_hook

    install_neuronx_cc_hook()
    n_cores = 8
    devices = jax.devices()[:n_cores]
    mesh = Mesh(np.asarray(devices), ("core",))
    sharding = jax.sharding.NamedSharding(mesh, PartitionSpec("core"))
    entries = []
    for nc in ncs:
        partition_name = nc.partition_id_tensor.name if nc.partition_id_tensor else None
        in_names, out_names, out_avals, zero_outs = [], [], [], []
        for alloc in nc.m.functions[0].allocations:
            if not isinstance(alloc, mybir.MemoryLocationSet):
                continue
            name = alloc.memorylocations[0].name
            if alloc.kind == "ExternalInput":
                if name != partition_name:
                    in_names.append(name)
            elif alloc.kind == "ExternalOutput":
                out_names.append(name)
                shape = tuple(alloc.tensor_shape)
                dtype = mybir.dt.np(alloc.dtype)
                out_avals.append(jax.core.ShapedArray(shape, dtype))
                zero_outs.append(np.zeros(shape, dtype))
        n_params = len(in_names)
        n_outs = len(out_avals)
        all_names = in_names + out_names + ([partition_name] if partition_name else [])

        def _make_body(nc=nc, partition_name=partition_name, out_avals=tuple(out_avals),
                       all_names=tuple(all_names), out_names=tuple(out_names)):
            def _body(*args):
                operands = list(args)
                if partition_name is not None:
                    operands.append(partition_id_tensor())
                return tuple(_bass_exec_p.bind(
                    *operands, out_avals=out_avals, in_names=all_names,
                    out_names=out_names, lowering_input_output_aliases=(),
                    sim_require_finite=True, sim_require_nnan=True, nc=nc))
            return _body

        donate = tuple(range(n_params, n_params + n_outs))
        sharded = jax.jit(
            shard_map(_make_body(), mesh=mesh,
                      in_specs=(PartitionSpec("core"),) * (n_params + n_outs),
                      out_specs=(PartitionSpec("core"),) * n_outs, check_rep=False),
            donate_argnums=donate, keep_unused=True)
        concat_in = [np.concatenate([np.asarray(in_maps[c][nm]) for c in range(n_cores)], axis=0)
                     for nm in in_names]
        dev_in = [jax.device_put(a, sharding) for a in concat_in]

        def fz(zero_outs=zero_outs):
            return [jax.device_put(np.zeros((n_cores * z.shape[0], *z.shape[1:]), z.dtype), sharding)
                    for z in zero_outs]
        entries.append(dict(sharded=sharded, dev_in=dev_in, fz=fz, times=[]))

    chunk = 12
    rounds = max(1, iters // chunk)
    for e in entries:
        for _ in range(warmup):
            jax.block_until_ready(e["sharded"](*e["dev_in"], *e["fz"]()))
    for r in range(rounds):
        for e in entries:
            zsets = [e["fz"]() for _ in range(chunk)]
            jax.block_until_ready(zsets)
            t0 = time.time()
            outs = [e["sharded"](*e["dev_in"], *zsets[i]) for i in range(chunk)]
            jax.block_until_ready(outs)
            e["times"].append((time.time() - t0) / chunk)
    out = []
    for e in entries:
        ts = sorted(e["times"])
        k = max(1, (len(ts) + 1) // 2)
        out.append(sum(ts[:k]) / k)   # mean of fastest half (drift-robust)
    return out

